# revision 24
# baseline (speedup 1.0000x reference)
"""TRN2 Bass kernel for the LSQ-quantized 2-layer MLP.

reference computation:
    wq1 = lsq_quant(w1, alpha1); wq2 = lsq_quant(w2, alpha2)   (tiny 256x256)
    h = relu(x @ wq1.T + b1)
    y = sigmoid(h @ wq2.T + b2)                                 x: [262144, 256] f32

Data-parallel over 8 NeuronCores (32768 tokens/core), no collectives.

Numerics strategy (device works entirely in fp8-e4m3 / f32-psum):
  * LSQ levels k = round(clip(w/a, -8, 7)) are small integers -> exact in
    fp8e4. fc1 stores k1/16 (still exact in fp8) so the stored
    h = relu(z1)/16 stays inside fp8 range (z1 std ~26, fp8 max 240 ->
    raw relu(z1) occasionally overflows to inf). Both layers run as
    DoubleRow fp8 matmuls (virtual K=256 in a single MM).
  * x is pre-quantized to fp8e4 on the host (~2.5% rms/elem); with
    sigmoid' <= 0.25 and z2 std ~0.08 the end-to-end l2 error stays
    ~2e-3, well under the 2e-2 gate.
  * Scales fold: y = 0.5 + 0.5*tanh((8*a1*a2)*z2' + b2/2). The device
    stores t = tanh(...) in fp8 (t is centered at 0, std ~0.04, which fp8
    holds to ~2.5% rel; sigmoid output itself hugs 0.5 where fp8 steps
    are 0.0625 -> unusable). The host applies the affine 0.5 + 0.5*t
    during the gather/unshard pass (dequantization only, no
    transcendental on host).

Device pipeline, per 512-token block (64 blocks/core), channel-major:
    sync DMA load xT fp8 (one 256 KiB load per block pair, 2 KiB/partition)
    fc1: 2 DoubleRow MMs (j=0,1) -> ph PSUM [128,1024] f32 (2 banks)
    relu: one DVE tensor_scalar max over the flat 1024 f32 -> hT fp8 SBUF
    fc2: 2 DoubleRow MMs -> py PSUM (2 banks)
    tanh: one ACT activation over the flat 1024 -> tT fp8 SBUF
    sync DMA store tT fp8 per block pair
Known-good details (measured on HW):
  * Elementwise APs must be flat 1D-free: a [2,512] free AP costs a second
    full ACT/DVE setup pass (~+300ns per op).
  * ph/py PSUM pools are double-buffered 2-bank tiles (8 banks exactly);
    fc1 of block b is emitted before fc2 of block b-1 so the PE FIFO
    never stalls on the DVE relu.
  * memset runs on the DVE: a gpsimd memset pays the ~6us Q7 IRAM load on
    the startup critical path.
  * A dummy tanh up front pulls the ~2.7us ACT table load off the
    critical path.
  * PE duty is ~65-70%, which sits at the HAM clock-gate bistable point:
    runs measure ~100us or ~113us depending on oscillation phase. Adding
    PE work (dummy MMs, split MMs) to force 8/8 measured strictly worse
    (108-120us median); the bare version keeps the best mode.

Steady state is paced by the DVE relu (~1.26us/block) with ACT tanh
~1.21us/block; the PE (4 DoubleRow MMs, ~0.9us warm) and DMA (~0.6us)
ride underneath. 2 MiB PSUM (8 banks) is what caps the elementwise
free-dim at 1024 and hence the ~1.26us floor.
"""

import numpy as np

import concourse.mybir as mybir
import concourse.tile as tile
from concourse import bacc
from concourse.bass import ts
from concourse.bass_utils import run_bass_kernel_spmd

N_CORES = 8
N_TOK = 262144
C = 256
TOK_PER_CORE = N_TOK // N_CORES  # 32768
T_BLK = 512
N_BLK = TOK_PER_CORE // T_BLK  # 64
N_PAIR = N_BLK // 2
P = 128

F32 = mybir.dt.float32
F8 = mybir.dt.float8e4

_program_cache = {}


def _build_program(use_b1: bool, use_b2: bool, s2: float):
    nc = bacc.Bacc("TRN2", target_bir_lowering=False, debug=False, num_devices=N_CORES)

    xt_d = nc.declare_dram_parameter("xt", [N_PAIR, P, 4 * T_BLK], F8, isOutput=False)
    wk_d = nc.declare_dram_parameter("wk", [P, 2, 4, P], F8, isOutput=False)
    if use_b1:
        b1s_d = nc.declare_dram_parameter("b1s", [P, 2], F32, isOutput=False)
    if use_b2:
        b2s_d = nc.declare_dram_parameter("b2s", [P, 2], F32, isOutput=False)
    yt_d = nc.declare_dram_parameter("yt", [N_PAIR, P, 4 * T_BLK], F8, isOutput=True)

    DR = mybir.MatmulPerfMode.DoubleRow
    Tanh = mybir.ActivationFunctionType.Tanh
    Relu = mybir.ActivationFunctionType.Relu
    # DVE (relu) paces the pipeline at ~1.26us/block vs ACT ~1.18; running
    # a couple of whole relus on ACT instead equalizes the two queues
    # (relu is filler in every ACT table set -> no table switch)
    ACT_RELU_BLOCKS = {21, 43}

    def as_kn(ap):
        # flat [P, 2*T_BLK] fp8 -> DoubleRow moving AP [P, 2, T_BLK]
        return ap.rearrange("p (i t) -> p i t", i=2)

    with tile.TileContext(nc) as tc:
        with (
            tc.tile_pool(name="const", bufs=1) as const_pool,
            tc.tile_pool(name="sb_xt", bufs=4) as sb_xt,
            tc.tile_pool(name="sb_ht", bufs=3) as sb_ht,
            tc.tile_pool(name="sb_yt", bufs=2) as sb_yt,
            tc.tile_pool(name="ps_h", bufs=2, space="PSUM") as ps_h,
            tc.tile_pool(name="ps_y", bufs=2, space="PSUM") as ps_y,
        ):
            wk = const_pool.tile([P, 2, 4, P], F8)
            nc.scalar.dma_start(wk[:], wk_d[:])
            if use_b1:
                b1s = const_pool.tile([P, 2], F32)
                nc.scalar.dma_start(b1s[:], b1s_d[:])
            if use_b2:
                b2s = const_pool.tile([P, 2], F32)
                nc.scalar.dma_start(b2s[:], b2s_d[:])

            warm = const_pool.tile([P, 2 * T_BLK], F8)
            # DVE memset: a gpsimd memset would pay the ~6us Q7 IRAM load
            # on the startup critical path
            nc.vector.memset(warm[:], 0.0)
            wscr = const_pool.tile([P, P], F8)
            # pull the tanh table load off the critical path
            nc.scalar.activation(wscr[:], warm[:, :P], Tanh, bias=0.0, scale=s2)

            xts = [None, None]
            hts = [None, None]
            phs = [None, None]
            yts = [None, None]
            for b in range(N_BLK + 1):
                if b < N_BLK:
                    if b % 2 == 0:
                        xt = sb_xt.tile([P, 4 * T_BLK], F8, tag="xt")
                        nc.sync.dma_start(xt[:], xt_d[b // 2])
                        xts[(b // 2) % 2] = xt
                    xt = xts[(b // 2) % 2]
                    xb = xt[:, (b % 2) * 2 * T_BLK : (b % 2 + 1) * 2 * T_BLK]
                    ph = ps_h.tile([P, 2 * T_BLK], F32, tag="ph")
                    if b == 0 and not use_b1:
                        # half-block the pipeline-fill path: halves every
                        # link of the serial fc1->relu chain while the
                        # PE is still cold
                        H = T_BLK // 2
                        ht0 = sb_ht.tile([P, 2 * T_BLK], F8, tag="ht")
                        for h in range(2):
                            for j in range(2):
                                nc.tensor.matmul(
                                    ph[:, j * T_BLK + h * H : j * T_BLK + (h + 1) * H],
                                    wk[:, :, j, :],
                                    as_kn(xb)[:, :, h * H : (h + 1) * H],
                                    start=True,
                                    stop=True,
                                    perf_mode=DR,
                                )
                            nc.vector.tensor_scalar_max(
                                as_kn(ht0[:])[:, :, h * H : (h + 1) * H],
                                as_kn(ph[:])[:, :, h * H : (h + 1) * H],
                                0.0,
                            )
                        hts[0] = ht0
                    else:
                        for j in range(2):
                            nc.tensor.matmul(
                                ph[:, ts(j, T_BLK)],
                                wk[:, :, j, :],
                                as_kn(xb),
                                start=True,
                                stop=True,
                                perf_mode=DR,
                            )
                    phs[b % 2] = ph

                if b >= 1:
                    c = b - 1
                    ht = hts[c % 2]
                    py = ps_y.tile([P, 2 * T_BLK], F32, tag="py")
                    if c == 0 and not use_b1:
                        H = T_BLK // 2
                        for h in range(2):
                            for j in range(2):
                                nc.tensor.matmul(
                                    py[:, j * T_BLK + h * H : j * T_BLK + (h + 1) * H],
                                    wk[:, :, 2 + j, :],
                                    as_kn(ht[:])[:, :, h * H : (h + 1) * H],
                                    start=True,
                                    stop=True,
                                    perf_mode=DR,
                                )
                    else:
                        for j in range(2):
                            nc.tensor.matmul(
                                py[:, ts(j, T_BLK)],
                                wk[:, :, 2 + j, :],
                                as_kn(ht[:]),
                                start=True,
                                stop=True,
                                perf_mode=DR,
                            )
                    if c % 2 == 0:
                        yt = sb_yt.tile([P, 4 * T_BLK], F8, tag="yt")
                        yts[(c // 2) % 2] = yt
                    yt = yts[(c // 2) % 2]
                    yb = yt[:, (c % 2) * 2 * T_BLK : (c % 2 + 1) * 2 * T_BLK]
                    if use_b2:
                        for j in range(2):
                            nc.scalar.activation(
                                yb.rearrange("p (j t) -> p j t", j=2)[:, j, :],
                                py[:, ts(j, T_BLK)],
                                Tanh,
                                bias=b2s[:, j : j + 1],
                                scale=s2,
                            )
                    else:
                        nc.scalar.activation(yb, py[:], Tanh, bias=0.0, scale=s2)
                    if c % 2 == 1:
                        nc.sync.dma_start(yt_d[c // 2], yt[:])

                if b < N_BLK:
                    if b == 0 and not use_b1:
                        continue  # block-0 relu already issued inline
                    # h_stored = relu(z1/16 [+ b1/(16 a1)]) in fp8
                    ph = phs[b % 2]
                    ht = sb_ht.tile([P, 2 * T_BLK], F8, tag="ht")
                    if use_b1:
                        for j in range(2):
                            nc.vector.tensor_scalar(
                                ht[:, ts(j, T_BLK)],
                                ph[:, ts(j, T_BLK)],
                                b1s[:, j : j + 1],
                                0.0,
                                mybir.AluOpType.add,
                                mybir.AluOpType.max,
                            )
                    elif b in ACT_RELU_BLOCKS:
                        nc.scalar.activation(
                            ht[:], ph[:], Relu, bias=0.0, scale=1.0
                        )
                    else:
                        nc.vector.tensor_scalar_max(ht[:], ph[:], 0.0)
                    hts[b % 2] = ht

    nc.compile()
    return nc


def _quantize_lsq_int(w: np.ndarray, alpha) -> tuple[np.ndarray, np.float32]:
    """Integer LSQ levels k = round(clip(w/a, -8, 7)) and effective scale a,
    replicating the reference forward numerics in np float32."""
    one = np.float32(1.0)
    g = one / np.sqrt(np.float32(w.size * 7))
    alpha = np.float32(alpha)
    a = np.float32(alpha * g) + np.float32(alpha * np.float32(one - g))
    t = np.clip((w / a).astype(np.float32), np.float32(-8.0), np.float32(7.0))
    r = (np.round(t) - t).astype(np.float32)
    q = (t + r).astype(np.float32)  # integer levels in [-8, 7]
    return q, a


def _prepare(x, w1, b1, alpha1, w2, b2, alpha2):
    import ml_dtypes

    f8 = ml_dtypes.float8_e4m3

    x = np.asarray(x, dtype=np.float32)
    w1 = np.asarray(w1, dtype=np.float32)
    w2 = np.asarray(w2, dtype=np.float32)
    b1 = np.asarray(b1, dtype=np.float32)
    b2 = np.asarray(b2, dtype=np.float32)

    k1, a1 = _quantize_lsq_int(w1, alpha1)
    k2, a2 = _quantize_lsq_int(w2, alpha2)
    k1 = k1 / np.float32(16.0)  # exact in fp8; keeps stored h in range

    # fc1 contraction channel c = 2p+i ; fc2 contraction channel c = i*128+p
    w1_pim = k1.T.reshape(P, 2, 2, P)  # [p, i, j, m]
    w2_pim = k2.T.reshape(2, P, 2, P).transpose(1, 0, 2, 3)  # [p, i, j, m]
    wk = np.concatenate([w1_pim, w2_pim], axis=2).astype(f8)
    wk = np.ascontiguousarray(wk)

    s2 = float(np.float32(8.0) * a1 * a2)
    use_b1 = bool(np.any(b1))
    use_b2 = bool(np.any(b2))
    key = (use_b1, use_b2, s2)
    if key not in _program_cache:
        _program_cache[key] = _build_program(use_b1, use_b2, s2)
    nc = _program_cache[key]

    in_maps = []
    for i in range(N_CORES):
        shard = x[i * TOK_PER_CORE : (i + 1) * TOK_PER_CORE]
        xt = np.ascontiguousarray(shard.T.astype(f8))  # [256, 32768] c=2p+i
        # -> [pair, p, (q, i, t)] so each block-pair is contiguous per partition
        xt = xt.reshape(P, 2, N_PAIR, 2, T_BLK).transpose(2, 0, 3, 1, 4)
        xt = np.ascontiguousarray(xt).reshape(N_PAIR, P, 4 * T_BLK)
        m = {"xt": xt, "wk": wk}
        if use_b1:
            m["b1s"] = np.ascontiguousarray(
                (b1 / (np.float32(16.0) * a1)).reshape(2, P).T
            )
        if use_b2:
            m["b2s"] = np.ascontiguousarray((b2 * np.float32(0.5)).reshape(2, P).T)
        in_maps.append(m)
    return nc, in_maps


def kernel(x, w1, b1, alpha1, w2, b2, alpha2):
    nc, in_maps = _prepare(x, w1, b1, alpha1, w2, b2, alpha2)
    res = run_bass_kernel_spmd(nc, in_maps, list(range(N_CORES)))
    outs = []
    for i in range(N_CORES):
        t = np.asarray(res.results[i]["yt"]).astype(np.float32)
        # [pair, p, q, i, t] -> [tok, i*128+p]
        t = t.reshape(N_PAIR, P, 2, 2, T_BLK).transpose(0, 2, 4, 3, 1)
        y = np.ascontiguousarray(t).reshape(TOK_PER_CORE, C)
        outs.append(y)
    out = np.concatenate(outs, axis=0)
    out *= np.float32(0.5)
    out += np.float32(0.5)
    return out


# revision 26
# speedup vs baseline: 1.1040x; 1.1040x over previous
"""TRN2 Bass kernel for the LSQ-quantized 2-layer MLP.

reference computation:
    wq1 = lsq_quant(w1, alpha1); wq2 = lsq_quant(w2, alpha2)   (tiny 256x256)
    h = relu(x @ wq1.T + b1)
    y = sigmoid(h @ wq2.T + b2)                                 x: [262144, 256] f32

Data-parallel over 8 NeuronCores (32768 tokens/core), no collectives.

Numerics strategy (device works entirely in fp8-e4m3 / f32-psum):
  * LSQ levels k = round(clip(w/a, -8, 7)) are small integers -> exact in
    fp8e4. fc1 stores k1/16 (still exact in fp8) so the stored
    h = relu(z1)/16 stays inside fp8 range (z1 std ~26, fp8 max 240 ->
    raw relu(z1) occasionally overflows to inf). Both layers run as
    DoubleRow fp8 matmuls (virtual K=256 in a single MM).
  * x is pre-quantized to fp8e4 on the host (~2.5% rms/elem); with
    sigmoid' <= 0.25 and z2 std ~0.08 the end-to-end l2 error stays
    ~2e-3, well under the 2e-2 gate.
  * Scales fold: y = 0.5 + 0.5*tanh((8*a1*a2)*z2' + b2/2). The device
    stores t = tanh(...) in fp8 (t is centered at 0, std ~0.04, which fp8
    holds to ~2.5% rel; sigmoid output itself hugs 0.5 where fp8 steps
    are 0.0625 -> unusable). The host applies the affine 0.5 + 0.5*t
    during the gather/unshard pass (dequantization only, no
    transcendental on host).

Device pipeline, per 512-token block (64 blocks/core), channel-major:
    sync DMA load xT fp8 (one 256 KiB load per block pair, 2 KiB/partition)
    fc1: 2 DoubleRow MMs (j=0,1) -> ph PSUM [128,1024] f32 (2 banks)
    relu: one DVE tensor_scalar max over the flat 1024 f32 -> hT fp8 SBUF
    fc2: 2 DoubleRow MMs -> py PSUM (2 banks)
    tanh: one ACT activation over the flat 1024 -> tT fp8 SBUF
    sync DMA store tT fp8 per block pair
Known-good details (measured on HW):
  * Elementwise APs must be flat 1D-free: a [2,512] free AP costs a second
    full ACT/DVE setup pass (~+300ns per op).
  * ph/py PSUM pools are double-buffered 2-bank tiles (8 banks exactly);
    fc1 of block b is emitted before fc2 of block b-1 so the PE FIFO
    never stalls on the DVE relu.
  * memset runs on the DVE: a gpsimd memset pays the ~6us Q7 IRAM load on
    the startup critical path.
  * A dummy tanh up front pulls the ~2.7us ACT table load off the
    critical path.
  * PE duty is ~65-70%, which sits at the HAM clock-gate bistable point:
    runs measure ~100us or ~113us depending on oscillation phase. Adding
    PE work (dummy MMs, split MMs) to force 8/8 measured strictly worse
    (108-120us median); the bare version keeps the best mode.

Steady state is paced by the DVE relu (~1.26us/block) with ACT tanh
~1.21us/block; the PE (4 DoubleRow MMs, ~0.9us warm) and DMA (~0.6us)
ride underneath. 2 MiB PSUM (8 banks) is what caps the elementwise
free-dim at 1024 and hence the ~1.26us floor.
"""

import numpy as np

import concourse.mybir as mybir
import concourse.tile as tile
from concourse import bacc
from concourse.bass import ts
from concourse.bass_utils import run_bass_kernel_spmd

N_CORES = 8
N_TOK = 262144
C = 256
TOK_PER_CORE = N_TOK // N_CORES  # 32768
T_BLK = 512
N_BLK = TOK_PER_CORE // T_BLK  # 64
N_PAIR = N_BLK // 2
P = 128

F32 = mybir.dt.float32
F8 = mybir.dt.float8e4

_program_cache = {}


def _build_program(use_b1: bool, use_b2: bool, s2: float):
    nc = bacc.Bacc("TRN2", target_bir_lowering=False, debug=False, num_devices=N_CORES)

    xt_d = nc.declare_dram_parameter("xt", [N_PAIR, P, 4 * T_BLK], F8, isOutput=False)
    wk_d = nc.declare_dram_parameter("wk", [P, 2, 4, P], F8, isOutput=False)
    if use_b1:
        b1s_d = nc.declare_dram_parameter("b1s", [P, 2], F32, isOutput=False)
    if use_b2:
        b2s_d = nc.declare_dram_parameter("b2s", [P, 2], F32, isOutput=False)
    yt_d = nc.declare_dram_parameter("yt", [N_PAIR, P, 4 * T_BLK], F8, isOutput=True)

    DR = mybir.MatmulPerfMode.DoubleRow
    Tanh = mybir.ActivationFunctionType.Tanh
    Relu = mybir.ActivationFunctionType.Relu
    # DVE (relu) paces the pipeline at ~1.26us/block vs ACT ~1.18; running
    # a couple of whole relus on ACT instead equalizes the two queues
    # (relu is filler in every ACT table set -> no table switch)
    ACT_RELU_BLOCKS = {21, 43}

    def as_kn(ap):
        # flat [P, 2*T_BLK] fp8 -> DoubleRow moving AP [P, 2, T_BLK]
        return ap.rearrange("p (i t) -> p i t", i=2)

    with tile.TileContext(nc) as tc:
        with (
            tc.tile_pool(name="const", bufs=1) as const_pool,
            tc.tile_pool(name="sb_xt", bufs=4) as sb_xt,
            tc.tile_pool(name="sb_ht", bufs=3) as sb_ht,
            tc.tile_pool(name="sb_yt", bufs=2) as sb_yt,
            tc.tile_pool(name="ps_h", bufs=2, space="PSUM") as ps_h,
            tc.tile_pool(name="ps_y", bufs=2, space="PSUM") as ps_y,
        ):
            wk = const_pool.tile([P, 2, 4, P], F8)
            nc.scalar.dma_start(wk[:], wk_d[:])
            if use_b1:
                b1s = const_pool.tile([P, 2], F32)
                nc.scalar.dma_start(b1s[:], b1s_d[:])
            if use_b2:
                b2s = const_pool.tile([P, 2], F32)
                nc.scalar.dma_start(b2s[:], b2s_d[:])

            warm = const_pool.tile([P, 2 * T_BLK], F8)
            # DVE memset: a gpsimd memset would pay the ~6us Q7 IRAM load
            # on the startup critical path
            nc.vector.memset(warm[:], 0.0)
            wscr = const_pool.tile([P, P], F8)
            # pull the tanh table load off the critical path
            nc.scalar.activation(wscr[:], warm[:, :P], Tanh, bias=0.0, scale=s2)

            xts = [None, None]
            hts = [None, None]
            phs = [None, None]
            yts = [None, None]
            for b in range(N_BLK + 1):
                if b < N_BLK:
                    if b % 2 == 0:
                        xt = sb_xt.tile([P, 4 * T_BLK], F8, tag="xt")
                        nc.sync.dma_start(xt[:], xt_d[b // 2])
                        xts[(b // 2) % 2] = xt
                    xt = xts[(b // 2) % 2]
                    xb = xt[:, (b % 2) * 2 * T_BLK : (b % 2 + 1) * 2 * T_BLK]
                    ph = ps_h.tile([P, 2 * T_BLK], F32, tag="ph")
                    if b == 0 and not use_b1:
                        # half-block the pipeline-fill path: halves every
                        # link of the serial fc1->relu chain while the
                        # PE is still cold
                        H = T_BLK // 2
                        ht0 = sb_ht.tile([P, 2 * T_BLK], F8, tag="ht")
                        for h in range(2):
                            for j in range(2):
                                nc.tensor.matmul(
                                    ph[:, j * T_BLK + h * H : j * T_BLK + (h + 1) * H],
                                    wk[:, :, j, :],
                                    as_kn(xb)[:, :, h * H : (h + 1) * H],
                                    start=True,
                                    stop=True,
                                    perf_mode=DR,
                                )
                            nc.vector.tensor_scalar_max(
                                as_kn(ht0[:])[:, :, h * H : (h + 1) * H],
                                as_kn(ph[:])[:, :, h * H : (h + 1) * H],
                                0.0,
                            )
                        hts[0] = ht0
                    else:
                        for j in range(2):
                            nc.tensor.matmul(
                                ph[:, ts(j, T_BLK)],
                                wk[:, :, j, :],
                                as_kn(xb),
                                start=True,
                                stop=True,
                                perf_mode=DR,
                            )
                    phs[b % 2] = ph

                if b >= 1:
                    c = b - 1
                    ht = hts[c % 2]
                    py = ps_y.tile([P, 2 * T_BLK], F32, tag="py")
                    if c == 0 and not use_b1:
                        H = T_BLK // 2
                        for h in range(2):
                            for j in range(2):
                                nc.tensor.matmul(
                                    py[:, j * T_BLK + h * H : j * T_BLK + (h + 1) * H],
                                    wk[:, :, 2 + j, :],
                                    as_kn(ht[:])[:, :, h * H : (h + 1) * H],
                                    start=True,
                                    stop=True,
                                    perf_mode=DR,
                                )
                    else:
                        for j in range(2):
                            nc.tensor.matmul(
                                py[:, ts(j, T_BLK)],
                                wk[:, :, 2 + j, :],
                                as_kn(ht[:]),
                                start=True,
                                stop=True,
                                perf_mode=DR,
                            )
                    if c % 2 == 0:
                        yt = sb_yt.tile([P, 4 * T_BLK], F8, tag="yt")
                        yts[(c // 2) % 2] = yt
                    yt = yts[(c // 2) % 2]
                    yb = yt[:, (c % 2) * 2 * T_BLK : (c % 2 + 1) * 2 * T_BLK]
                    if use_b2:
                        for j in range(2):
                            nc.scalar.activation(
                                yb.rearrange("p (j t) -> p j t", j=2)[:, j, :],
                                py[:, ts(j, T_BLK)],
                                Tanh,
                                bias=b2s[:, j : j + 1],
                                scale=s2,
                            )
                    else:
                        nc.scalar.activation(yb, py[:], Tanh, bias=0.0, scale=s2)
                    if c % 2 == 1:
                        nc.sync.dma_start(yt_d[c // 2], yt[:])

                if b < N_BLK:
                    if b == 0 and not use_b1:
                        continue  # block-0 relu already issued inline
                    # h_stored = relu(z1/16 [+ b1/(16 a1)]) in fp8
                    ph = phs[b % 2]
                    ht = sb_ht.tile([P, 2 * T_BLK], F8, tag="ht")
                    if use_b1:
                        for j in range(2):
                            nc.vector.tensor_scalar(
                                ht[:, ts(j, T_BLK)],
                                ph[:, ts(j, T_BLK)],
                                b1s[:, j : j + 1],
                                0.0,
                                mybir.AluOpType.add,
                                mybir.AluOpType.max,
                            )
                    elif b in ACT_RELU_BLOCKS:
                        nc.scalar.activation(
                            ht[:], ph[:], Relu, bias=0.0, scale=1.0
                        )
                    else:
                        nc.vector.tensor_scalar_max(ht[:], ph[:], 0.0)
                    hts[b % 2] = ht

    nc.compile()
    return nc


def _quantize_lsq_int(w: np.ndarray, alpha) -> tuple[np.ndarray, np.float32]:
    """Integer LSQ levels k = round(clip(w/a, -8, 7)) and effective scale a,
    replicating the reference forward numerics in np float32."""
    one = np.float32(1.0)
    g = one / np.sqrt(np.float32(w.size * 7))
    alpha = np.float32(alpha)
    a = np.float32(alpha * g) + np.float32(alpha * np.float32(one - g))
    t = np.clip((w / a).astype(np.float32), np.float32(-8.0), np.float32(7.0))
    r = (np.round(t) - t).astype(np.float32)
    q = (t + r).astype(np.float32)  # integer levels in [-8, 7]
    return q, a


def _prepare(x, w1, b1, alpha1, w2, b2, alpha2):
    import ml_dtypes

    f8 = ml_dtypes.float8_e4m3

    x = np.asarray(x, dtype=np.float32)
    w1 = np.asarray(w1, dtype=np.float32)
    w2 = np.asarray(w2, dtype=np.float32)
    b1 = np.asarray(b1, dtype=np.float32)
    b2 = np.asarray(b2, dtype=np.float32)

    k1, a1 = _quantize_lsq_int(w1, alpha1)
    k2, a2 = _quantize_lsq_int(w2, alpha2)
    k1 = k1 / np.float32(16.0)  # exact in fp8; keeps stored h in range

    # fc1 contraction channel c = 2p+i ; fc2 contraction channel c = i*128+p
    w1_pim = k1.T.reshape(P, 2, 2, P)  # [p, i, j, m]
    w2_pim = k2.T.reshape(2, P, 2, P).transpose(1, 0, 2, 3)  # [p, i, j, m]
    wk = np.concatenate([w1_pim, w2_pim], axis=2).astype(f8)
    wk = np.ascontiguousarray(wk)

    s2 = float(np.float32(8.0) * a1 * a2)
    use_b1 = bool(np.any(b1))
    use_b2 = bool(np.any(b2))
    key = (use_b1, use_b2, s2)
    if key not in _program_cache:
        _program_cache[key] = _build_program(use_b1, use_b2, s2)
    nc = _program_cache[key]

    in_maps = []
    for i in range(N_CORES):
        shard = x[i * TOK_PER_CORE : (i + 1) * TOK_PER_CORE]
        xt = np.ascontiguousarray(shard.T.astype(f8))  # [256, 32768] c=2p+i
        # -> [pair, p, (q, i, t)] so each block-pair is contiguous per partition
        xt = xt.reshape(P, 2, N_PAIR, 2, T_BLK).transpose(2, 0, 3, 1, 4)
        xt = np.ascontiguousarray(xt).reshape(N_PAIR, P, 4 * T_BLK)
        m = {"xt": xt, "wk": wk}
        if use_b1:
            m["b1s"] = np.ascontiguousarray(
                (b1 / (np.float32(16.0) * a1)).reshape(2, P).T
            )
        if use_b2:
            m["b2s"] = np.ascontiguousarray((b2 * np.float32(0.5)).reshape(2, P).T)
        in_maps.append(m)
    return nc, in_maps


def kernel(x, w1, b1, alpha1, w2, b2, alpha2):
    nc, in_maps = _prepare(x, w1, b1, alpha1, w2, b2, alpha2)
    res = run_bass_kernel_spmd(nc, in_maps, list(range(N_CORES)))
    outs = []
    for i in range(N_CORES):
        t = np.asarray(res.results[i]["yt"]).astype(np.float32)
        # [pair, p, q, i, t] -> [tok, i*128+p]
        t = t.reshape(N_PAIR, P, 2, 2, T_BLK).transpose(0, 2, 4, 3, 1)
        y = np.ascontiguousarray(t).reshape(TOK_PER_CORE, C)
        outs.append(y)
    out = np.concatenate(outs, axis=0)
    out *= np.float32(0.5)
    out += np.float32(0.5)
    return out
